# revision 39
# baseline (speedup 1.0000x reference)
"""Causal multi-head attention on 8 Trainium2 NeuronCores.

Problem: x[4,2048,1024] @ w_qkv[1024,3072] -> causal MHA (16 heads, hd=64) -> @ w_out.

Sharding: batch (4) x head-group (2 x 8 heads) = 8 cores. Each core:
  phase 1: QKV projection for its batch + its 8 heads (Q^T,K^T transposed [ch,t],
           V natural [t,ch] with a ones column per head for the softmax denom).
  phase 2: causal attention per HEAD PAIR (2t, 2t+1): the two heads sit in
           partitions 0-63 / 64-127 of the same Q^T/K^T tile, so their K=64
           S^T matmuls are emitted back-to-back with tile_position (0,0)/(64,0)
           and run CONCURRENTLY in disjoint row-halves of the PE array.
           exp on ACT, 0/1 mask multiply on diagonal blocks, attn^T + denom
           accumulated via lhsT=[V|1], reciprocal + partition_broadcast + DVE mul.
  phase 3: partial output projection; host sums the two partials per batch.

All matmuls bf16 (fp8 fails the 2e-2 gate: sim rel-err 2-4e-2).
"""
import sys

if "/opt/trn_rl_repo" not in sys.path:
    sys.path.insert(0, "/opt/trn_rl_repo")

import ml_dtypes
import numpy as np

import concourse.tile as tile
from concourse import bacc, mybir
from concourse.bass import broadcast_tensor_aps as bass_broadcast
from concourse.bass_utils import run_bass_kernel_spmd

F32 = mybir.dt.float32
BF16 = mybir.dt.bfloat16
EXP = mybir.ActivationFunctionType.Exp

B, T, C, H = 4, 2048, 1024, 16
HD = C // H              # 64
HPC = 8                  # heads per core
CPC = HPC * HD           # 512 channels per core
NCHUNK = C // 128        # 8 contraction chunks of 128
NQ = 4                   # t-quarters (512 each) for phase-1 x streaming
TQ = T // NQ             # 512
NKB = T // 128           # 16 key blocks
NCT = CPC // 128         # 4 c'-tiles per projection (q and k each)

_NC_CACHE = None


def _build_nc():
    """Build the SPMD program (identical on all 8 cores)."""
    nc = bacc.Bacc()

    wqkv = nc.dram_tensor("wqkv", [NCHUNK, 128, 3 * CPC], BF16, kind="ExternalInput")
    xq = nc.dram_tensor("xq", [NQ, NCHUNK, 128, TQ], BF16, kind="ExternalInput")
    wo = nc.dram_tensor("wo", [NCT, 128, C], BF16, kind="ExternalInput")
    maskneg = nc.dram_tensor("maskneg", [128, 128], BF16, kind="ExternalInput")
    out = nc.dram_tensor("out", [T, C], F32, kind="ExternalOutput")

    with tile.TileContext(nc) as tc, \
         tc.tile_pool(name="pers", bufs=1) as pers, \
         tc.tile_pool(name="xpool", bufs=2) as xpool, \
         tc.tile_pool(name="epool", bufs=8) as epool, \
         tc.tile_pool(name="npool", bufs=4) as npool, \
         tc.tile_pool(name="opool", bufs=4) as opool, \
         tc.tile_pool(name="psum", bufs=1, space="PSUM") as psum:
        # persistent SBUF
        qkt = [pers.tile([128, T], BF16, name=f"qkt{i}") for i in range(2 * NCT)]
        vsb = pers.tile([128, NKB * (CPC + HPC)], BF16, name="vsb")  # 16 x (8 x 65)
        atn = [pers.tile([128, T], BF16, name=f"atn{i}") for i in range(NCT)]
        mask_sb = pers.tile([128, 128], BF16, name="mask_sb")
        nc.sync.dma_start(mask_sb[:], maskneg[:, :])
        # warm-up operand first on the DVE queue so warm-up matmuls start asap
        warm = pers.tile([128, 512], BF16, name="warm")
        nc.vector.memset(warm[:], 0.125)
        # ones columns of [V|1]: memset f32 staging, strided DVE copy (casts to bf16)
        ones_sb = pers.tile([128, NKB * HPC], F32, name="ones_sb")
        nc.vector.memset(ones_sb[:], 1.0)
        nc.vector.tensor_copy(
            vsb.rearrange("p (t h e) -> p (t h) e", h=HPC, e=HD + 1)[:, :, HD:HD + 1],
            ones_sb[:, :, None],
        )
        w_sb = [None] * NCHUNK

        def load_w(c, eng=None):
            wt = pers.tile([128, 3 * CPC], BF16, name=f"w{c}", uniquify=False)
            (eng or nc.sync).dma_start(wt[:], wqkv[c])
            w_sb[c] = wt
        wo_sb = []

        xt_cur = [None] * NCHUNK

        def load_x(tq):
            for c in range(NCHUNK):
                x_t = xpool.tile([128, TQ], BF16, name=f"x{c}", tag=f"x{c}")
                nc.sync.dma_start(x_t[:], xq[tq, c])
                xt_cur[c] = x_t

        def proj_unit(tq, g, xt=None):
            """One projection PSUM group: g in 0..11 (8 QK tiles + 4 V tiles)."""
            if xt is None:
                xt = list(xt_cur)
            if g < 2 * NCT:
                ps = psum.tile([128, TQ], F32, name="psqk", tag="S", bufs=3)
                for c in range(NCHUNK):
                    nc.tensor.matmul(
                        ps[:], w_sb[c][:, 128 * g:128 * (g + 1)], xt[c][:],
                        start=(c == 0), stop=(c == NCHUNK - 1),
                    )
                # drain on ACT for early quarters (ACT idles there, DVE is the
                # local wall); quarter-3 drains stay on DVE since ACT
                # saturates with j=3 exps
                if tq < 3:
                    nc.scalar.copy(qkt[g][:, TQ * tq:TQ * (tq + 1)], ps[:])
                else:
                    nc.vector.tensor_copy(qkt[g][:, TQ * tq:TQ * (tq + 1)], ps[:])
            else:
                vt = g - 2 * NCT
                ps = psum.tile([128, CPC], F32, name="psv", tag="S", bufs=3)
                for c in range(NCHUNK):
                    nc.tensor.matmul(
                        ps[:], xt[c][:, 128 * vt:128 * (vt + 1)],
                        w_sb[c][:, 2 * CPC:3 * CPC],
                        start=(c == 0), stop=(c == NCHUNK - 1),
                    )
                ti = tq * (TQ // 128) + vt
                dst = vsb[:, (CPC + HPC) * ti:(CPC + HPC) * (ti + 1)]
                nc.vector.tensor_copy(
                    dst.rearrange("p (h e) -> p h e", e=HD + 1)[:, :, 0:HD],
                    ps.rearrange("p (h e) -> p h e", e=HD),
                )

        def quarter_pipeline(j, fillers=None):
            """Causal attention for all 4 head pairs at query tile j, as one
            continuous pipeline: the S/exp stream runs LAG kb-steps ahead of
            the A stream ACROSS chain boundaries, so a chain's trailing A
            matmuls interleave with the next chain's leading S matmuls and the
            PE queue never drains at a boundary.

            Per kb step both heads' K=64 S matmuls write the SAME [128,1024]
            PSUM tile (head0 cols 0-511 / tile rows (0,0), head1 cols 512+ /
            rows (64,0)). Sharing one tile is what makes them run concurrently:
            the pool-rotation wait attaches only to the first matmul, so the
            pair sits back-to-back on the PE queue and overlaps in the array
            (~2x S throughput; separate tiles serialize).
            """
            q0 = 512 * j
            nkb = 4 * j + 4
            cols = []
            for kb in range(nkb):
                col0 = 0 if kb < 4 * j else 128 * (kb - 4 * j)
                cols.append((col0, 512 - col0))
            seq = [(t, kb) for t in range(NCT) for kb in range(nkb)]
            ees = {}
            pas = {}

            def emit_s(t, kb):
                col0, n = cols[kb]
                Q0 = qkt[t][0:HD, :]
                Q1 = qkt[t][HD:128, :]
                K0 = qkt[NCT + t][0:HD, :]
                K1 = qkt[NCT + t][HD:128, :]
                ss = psum.tile([128, 1024], F32, name="ss", tag="S", bufs=3)
                ee = epool.tile([128, 1024], BF16, name="ee", tag="E")
                nc.tensor.matmul(
                    ss[:, 0:n], K0[:, 128 * kb:128 * (kb + 1)],
                    Q0[:, q0 + col0:q0 + 512],
                    start=True, stop=True, skip_group_check=True,
                )
                nc.tensor.matmul(
                    ss[:, 512:512 + n], K1[:, 128 * kb:128 * (kb + 1)],
                    Q1[:, q0 + col0:q0 + 512],
                    start=True, stop=True, skip_group_check=True,
                )
                if n == 512:
                    nc.scalar.activation(ee[:], ss[:], EXP, scale=0.125)
                else:
                    # one exp for both heads via a 3D AP (stride-512 pair dim)
                    sv = ss[:].rearrange("p (a b) -> p a b", a=2, b=512)[:, :, 0:n]
                    ev = ee[:].rearrange("p (a b) -> p a b", a=2, b=512)[:, :, 0:n]
                    nc.scalar.activation(ev, sv, EXP, scale=0.125)
                if kb >= 4 * j:  # zero the masked (future) triangle post-exp
                    # one mul for both heads' diagonal 128-blocks; the mask AP
                    # broadcasts (stride 0) over the pair dim
                    ev = ee[:].rearrange("p (a b) -> p a b", a=2, b=512)[:, :, 0:128]
                    evb, mvb = bass_broadcast(ev, mask_sb[:, None, :])
                    nc.vector.tensor_mul(evb, evb, mvb)
                ees[(t, kb)] = ee

            def emit_a(t, kb):
                col0, n = cols[kb]
                ee = ees.pop((t, kb))
                if kb == 0:
                    pas[t] = [psum.tile([HD + 1, 512], F32, name=f"pa{i}",
                                        tag="A", bufs=2) for i in range(2)]
                pa = pas[t]
                for i, off in ((0, 0), (1, 512)):
                    h = 2 * t + i
                    nc.tensor.matmul(
                        pa[i][:, col0:512],
                        vsb[:, (CPC + HPC) * kb + (HD + 1) * h:
                             (CPC + HPC) * kb + (HD + 1) * (h + 1)],
                        ee[:, off:off + n],
                        start=(kb == 0), stop=(kb == nkb - 1),
                        skip_group_check=True,
                    )
                if kb == nkb - 1:
                    norm(t)

            def norm(t):
                # drain pa to SBUF right away (frees the 2 PSUM banks for the
                # next chain), then normalize from SBUF. NOTE: the reciprocal
                # input must sit at partition 0 — cross-partition SBUF->SBUF
                # ops silently corrupt on HW; only the PSUM->SBUF copy path
                # supports a partition offset.
                pa = pas.pop(t)
                for i in range(2):
                    r0 = HD * i
                    den = npool.tile([1, 512], F32, name="den", tag="den")
                    nc.vector.tensor_copy(den[:], pa[i][HD:HD + 1, :])
                    pc = npool.tile([HD, 512], F32, name="pc", tag="pc")
                    nc.vector.tensor_copy(pc[:], pa[i][0:HD, :])
                    rec = npool.tile([1, 512], F32, name="rec", tag="rec")
                    nc.vector.reciprocal_approx_fast(rec[:], den[:])
                    bc = npool.tile([HD, 512], F32, name="bc", tag="bc")
                    nc.gpsimd.partition_broadcast(bc[:], rec[:])
                    nc.vector.tensor_mul(
                        atn[t][r0:r0 + HD, q0:q0 + 512], pc[:], bc[:]
                    )

            LAG = 5
            nsl = len(seq) + LAG
            rate = (len(fillers) / nsl) if fillers else 0.0
            acc = 0.0
            for i in range(nsl):
                if i < len(seq):
                    emit_s(*seq[i])
                # filler lands between the S pair and the A pair: the A often
                # waits on exp/mask, so the filler keeps the PE queue moving
                if fillers:
                    acc += rate
                    while fillers and acc >= 1.0:
                        fillers.pop(0)()
                        acc -= 1.0
                if i >= LAG:
                    emit_a(*seq[i - LAG])
                if i == 2 * nkb and j + 2 < NQ:
                    load_x(j + 2)

        def outproj_unit(tq, g):
            """One output tile [t128, 512]: g in 0..7 (4 t-tiles x 2 col halves)."""
            tt = tq * 4 + g // 2
            jj = g % 2
            ps = psum.tile([128, 512], F32, name="po", tag="S", bufs=3)
            for cc in range(NCT):
                nc.tensor.matmul(
                    ps[:], atn[cc][:, 128 * tt:128 * (tt + 1)],
                    wo_sb[cc][:, 512 * jj:512 * (jj + 1)],
                    start=(cc == 0), stop=(cc == NCT - 1),
                )
            oc = opool.tile([128, 512], F32, name="oc", tag="oc")
            # quarter-1 output tiles drain during the j=2 window where ACT has
            # slack and DVE is contended; the rest stay on DVE (ACT saturates
            # in the j=3 window)
            if tq == 1:
                nc.scalar.copy(oc[:], ps[:])
            else:
                nc.vector.tensor_copy(oc[:], ps[:])
            nc.sync.dma_start(
                out[128 * tt:128 * (tt + 1), 512 * jj:512 * (jj + 1)], oc[:]
            )

        # ---- pipelined schedule ----
        # warm the HAM clock gate with dummy matmuls while startup DMAs land
        wps = psum.tile([128, 512], F32, name="wps", tag="S", bufs=3)
        for _ in range(10):
            nc.tensor.matmul(wps[:], warm[:, 0:128], warm[:],
                             start=True, stop=True, skip_group_check=True)
        for c in range(NCHUNK):
            x_t = xpool.tile([128, TQ], BF16, name=f"x{c}", tag=f"x{c}")
            nc.sync.dma_start(x_t[:], xq[0, c])
            xt_cur[c] = x_t
            load_w(c)
        for cc in range(NCT):
            wt = pers.tile([128, C], BF16, name=f"wo{cc}")
            nc.sync.dma_start(wt[:], wo[cc])
            wo_sb.append(wt)
        # emit only the proj(0) units that gate chain (t=0, j=0) — Q/K tiles
        # for t=0 plus all V units; the rest overlap the j=0 chains as fillers
        xt0 = list(xt_cur)
        for g in (0, 4, 8, 9, 10, 11):
            proj_unit(0, g, xt0)
        load_x(1)
        for tq in range(1, NQ + 1):
            j = tq - 1
            fillers = []
            if tq == 1:
                # remaining proj(0) Q/K tiles, in chain order (t=1,2,3)
                for t in range(1, NCT):
                    fillers.append((lambda g=t: proj_unit(0, g, xt0)))
                    fillers.append((lambda g=4 + t: proj_unit(0, g, xt0)))
            if tq < NQ:
                xts = list(xt_cur)
                for g in range(12):
                    fillers.append((lambda tq=tq, g=g, xts=xts: proj_unit(tq, g, xts)))
            # outproj(0) is deferred to the last (exp-heaviest) quarter so the
            # PE has filler while ACT streams the big j=3 exps
            if tq == 3:
                for g in range(8):
                    fillers.append((lambda g=g: outproj_unit(1, g)))
            if tq == NQ:
                for g in range(8):
                    fillers.append((lambda g=g: outproj_unit(0, g)))
                    fillers.append((lambda g=g: outproj_unit(2, g)))
            quarter_pipeline(j, fillers)
            if tq == NQ:
                for g in range(8):
                    outproj_unit(j, g)
                    if fillers:
                        fillers.pop(0)()
            while fillers:
                fillers.pop(0)()
    nc.finalize()
    return nc


def _prep_inputs(x, w_qkv, w_out):
    """Shard + pack host-side: returns in_maps for cores 0..7 (core = 2*b + g)."""
    in_maps = []
    maskneg = np.where(
        np.arange(128)[None, :] >= np.arange(128)[:, None], 1.0, 0.0
    ).astype(ml_dtypes.bfloat16)
    for b in range(B):
        xT = np.ascontiguousarray(x[b].T)  # [C, T]
        xq_bf = np.ascontiguousarray(
            xT.reshape(NCHUNK, 128, NQ, TQ).transpose(2, 0, 1, 3)
        ).astype(ml_dtypes.bfloat16)  # [NQ, NCHUNK, 128, TQ]
        for g in range(2):
            wq = w_qkv[:, CPC * g:CPC * (g + 1)]
            wk = w_qkv[:, C + CPC * g:C + CPC * (g + 1)]
            wv = w_qkv[:, 2 * C + CPC * g:2 * C + CPC * (g + 1)]
            wqkv_pack = np.concatenate([wq, wk, wv], axis=1).reshape(
                NCHUNK, 128, 3 * CPC
            )
            wo_pack = np.ascontiguousarray(
                w_out[CPC * g:CPC * (g + 1), :].reshape(NCT, 128, C)
            )
            in_maps.append({
                "wqkv": np.ascontiguousarray(wqkv_pack).astype(ml_dtypes.bfloat16),
                "xq": xq_bf,
                "wo": wo_pack.astype(ml_dtypes.bfloat16),
                "maskneg": maskneg,
            })
    return in_maps


def run(x, w_qkv, w_out, trace=False, trace_cores=None):
    global _NC_CACHE
    if _NC_CACHE is None:
        _NC_CACHE = _build_nc()
    in_maps = _prep_inputs(x, w_qkv, w_out)
    res = run_bass_kernel_spmd(
        _NC_CACHE, in_maps, list(range(8)),
        trace=trace, trace_cores=trace_cores,
    )
    outs = [res.results[i]["out"] for i in range(8)]
    full = np.empty((B, T, C), np.float32)
    for b in range(B):
        full[b] = outs[2 * b] + outs[2 * b + 1]
    return full, res


def kernel(x, w_qkv, w_out):
    x = np.asarray(x, np.float32)
    w_qkv = np.asarray(w_qkv, np.float32)
    w_out = np.asarray(w_out, np.float32)
    full, _ = run(x, w_qkv, w_out)
    return full
